# revision 6
# baseline (speedup 1.0000x reference)
"""YOLO-style loss (nn_Loss_52175262712573) on 8 Trainium2 NeuronCores.

Strategy: pure data parallel. The loss is a sum of independent per-(batch,
cell) "row" contributions; each row is 30 contiguous f32 channels
[b0: x,y,w,h,conf | b1: x,y,w,h,conf | 20 class scores]. We flatten
(batch, S, S) -> 802,816 rows, shard 100,352 rows per core, lay them out
as [128 partitions, 784 rows, 30 ch] per core, and stream 4 chunks of 196
rows/partition through SBUF. Each chunk produces two per-partition partial
sums (masked obj losses, noobj loss) via fused accumulate ops; the host
sums the 8x[128,8] outputs and divides by the global batch.

Self-contained: only needs numpy + the concourse (Bass/Tile) stack that is
installed on the machine.
"""

import numpy as np

import concourse.bass as bass
import concourse.mybir as mybir
import concourse.tile as tile
from concourse import bacc
from concourse.bass_utils import run_bass_kernel_spmd

F32 = mybir.dt.float32
ALU = mybir.AluOpType
ACT = mybir.ActivationFunctionType

# Problem constants (hardcoded per contract).
S = 14
NCH = 30
NB = 4096
NCORES = 8
P = 128                      # SBUF partitions
ROWS_PER_CORE = NB * S * S // NCORES      # 100352
RPP = ROWS_PER_CORE // P                  # 784 rows per partition
R = 196                                   # rows per chunk per partition
NCHUNK = RPP // R                         # 4
CHUNK_F = R * NCH                         # 5880 f32 per partition per chunk


def build_loss_kernel(tc, out_ap, pred_ap, targ_ap, ctx):
    """Emit the per-core loss kernel into TileContext `tc`.

    pred_ap/targ_ap: DRAM [128, RPP*30] f32 (rows of 30 channels).
    out_ap: DRAM [128, 2*NCHUNK] f32. out[:, 2k] = sum_rows m*(5*(lxy+lwh)
    + lobj + lclass); out[:, 2k+1] = sum_rows 0.5*(1-m)*(u0^2+u1^2).
    """
    nc = tc.nc
    pool_in = ctx.enter_context(tc.tile_pool(name="inp", bufs=2))
    tmp1 = ctx.enter_context(tc.tile_pool(name="tmp1", bufs=1))
    tmp2 = ctx.enter_context(tc.tile_pool(name="tmp2", bufs=2))
    pool_out = ctx.enter_context(tc.tile_pool(name="outp", bufs=1))

    out_sb = pool_out.tile([P, 2 * NCHUNK], F32)

    vec = nc.vector
    sca = nc.scalar

    for k in range(NCHUNK):
        Pt = pool_in.tile([P, CHUNK_F], F32, tag="P")
        Tt = pool_in.tile([P, CHUNK_F], F32, tag="T")
        nc.sync.dma_start(Pt[:], pred_ap[:, k * CHUNK_F:(k + 1) * CHUNK_F])
        nc.sync.dma_start(Tt[:], targ_ap[:, k * CHUNK_F:(k + 1) * CHUNK_F])

        P3 = Pt[:].rearrange("p (r c) -> p r c", c=NCH)
        T3 = Tt[:].rearrange("p (r c) -> p r c", c=NCH)
        Pb = P3[:, :, 0:10].rearrange("p r (b k) -> p r b k", k=5)
        Tb = T3[:, :, 0:10].rearrange("p r (b k) -> p r b k", k=5)
        P_xy4 = Pb[:, :, :, 0:2]          # [p,R,2,2]
        P_wh4 = Pb[:, :, :, 2:4]
        P_cf = Pb[:, :, :, 4]             # [p,R,2]
        T_xy0 = Tb[:, :, 0, 0:2]          # [p,R,2] (iou target = box 0)
        T_wh0 = Tb[:, :, 0, 2:4]
        T_xy4 = Tb[:, :, :, 0:2]
        T_wh4 = Tb[:, :, :, 2:4]
        T_m = T3[:, :, 4]                 # [p,R] obj mask (exactly 0/1)
        P_cls = P3[:, :, 10:30]
        T_cls = T3[:, :, 10:30]

        def t4(tag, bufs=1, pool=None):
            t = (pool or tmp1).tile([P, R * 4], F32, tag=tag, name=tag)
            return t, t[:].rearrange("p (r b k) -> p r b k", b=2, k=2)

        def t2(tag, bufs=1, pool=None):
            t = (pool or tmp1).tile([P, R * 2], F32, tag=tag, name=tag)
            return t, t[:].rearrange("p (r b) -> p r b", b=2)

        def t1(tag, pool=None):
            t = (pool or tmp1).tile([P, R], F32, tag=tag, name=tag)
            return t[:]

        # --- IoU of each pred box vs target box 0 (coords scaled by S) ---
        _, hP = t4("hP", pool=tmp2)        # (S/2)*wh of pred boxes
        sca.activation(hP, P_wh4, ACT.Copy, bias=0.0, scale=S / 2.0)
        _, hT = t2("hT", pool=tmp2)        # (S/2)*wh of target box 0
        sca.activation(hT, T_wh0, ACT.Copy, bias=0.0, scale=S / 2.0)

        _, dxyI = t4("dxyI")               # center offsets vs target box 0
        for b in range(2):
            vec.tensor_tensor(dxyI[:, :, b, :], P_xy4[:, :, b, :], T_xy0,
                              op=ALU.subtract)
        _, adxy2 = t4("adxy2", pool=tmp2)  # |dc|
        sca.activation(adxy2, dxyI, ACT.Abs, bias=0.0, scale=1.0)

        _, hsum = t4("hsum")
        _, wmin = t4("wmin")
        for b in range(2):
            vec.tensor_tensor(hsum[:, :, b, :], hP[:, :, b, :], hT, op=ALU.add)
            vec.tensor_tensor(wmin[:, :, b, :], hP[:, :, b, :], hT, op=ALU.min)
        _, o1 = t4("o1")
        vec.tensor_tensor(o1, hsum, adxy2, op=ALU.subtract)
        # overlap*2S = min(hp+ht-|2dc|... all scaled): w = min(2*wmin, o1)
        _, w = t4("w")
        vec.scalar_tensor_tensor(w, wmin, 2.0, o1, op0=ALU.mult, op1=ALU.min)
        vec.tensor_scalar(w, w, 0.0, None, op0=ALU.max)   # relu in place

        _, inter = t2("inter")             # 4*S^2 * intersection
        vec.tensor_tensor(inter, w[:, :, :, 0], w[:, :, :, 1], op=ALU.mult)
        _, areap = t2("areap")             # S^2/4 * pred area
        vec.tensor_tensor(areap, hP[:, :, :, 0], hP[:, :, :, 1], op=ALU.mult)
        areat = t1("areat")
        vec.tensor_tensor(areat, hT[:, :, 0], hT[:, :, 1], op=ALU.mult)
        _, asum = t2("asum")
        for b in range(2):
            vec.tensor_tensor(asum[:, :, b], areap[:, :, b], areat, op=ALU.add)
        _, den = t2("den")                 # 4*S^2 * union
        vec.scalar_tensor_tensor(den, asum, 4.0, inter,
                                 op0=ALU.mult, op1=ALU.subtract)
        _, rden = t2("rden")
        vec.reciprocal(rden, den)
        _, iou2 = t2("iou2")
        vec.tensor_tensor(iou2, inter, rden, op=ALU.mult)

        sel = t1("sel")                    # 1.0 iff box1 is responsible
        vec.tensor_tensor(sel, iou2[:, :, 1], iou2[:, :, 0], op=ALU.is_gt)
        mxiou = t1("mxiou")
        vec.tensor_tensor(mxiou, iou2[:, :, 0], iou2[:, :, 1], op=ALU.max)

        # --- per-box coord/obj losses ---
        _, dxyL = t4("dxyL")               # pred box b vs target box b
        vec.tensor_tensor(dxyL, P_xy4, T_xy4, op=ALU.subtract)
        _, sP = t4("sP", pool=tmp2)
        sca.activation(sP, P_wh4, ACT.Sqrt)
        _, sT = t4("sT", pool=tmp2)
        sca.activation(sT, T_wh4, ACT.Sqrt)
        _, dwq = t4("dwq")
        vec.tensor_tensor(dwq, sP, sT, op=ALU.subtract)
        _, du = t2("du")
        for b in range(2):
            vec.tensor_tensor(du[:, :, b], P_cf[:, :, b], mxiou,
                              op=ALU.subtract)
        sca.activation(dxyL, dxyL, ACT.Square)
        sca.activation(dwq, dwq, ACT.Square)
        sca.activation(du, du, ACT.Square)

        _, s1 = t2("s1")
        vec.tensor_tensor(s1, dxyL[:, :, :, 0], dxyL[:, :, :, 1], op=ALU.add)
        _, s2 = t2("s2")
        vec.tensor_tensor(s2, dwq[:, :, :, 0], dwq[:, :, :, 1], op=ALU.add)
        _, s12 = t2("s12")
        vec.tensor_tensor(s12, s1, s2, op=ALU.add)
        _, cb = t2("cb")                   # 5*(lxy+lwh) + lobj, per box
        vec.scalar_tensor_tensor(cb, s12, 5.0, du, op0=ALU.mult, op1=ALU.add)
        c = t1("c")                        # responsible box's loss
        vec.tensor_copy(c, cb[:, :, 0])
        vec.copy_predicated(c, sel.bitcast(mybir.dt.int32), cb[:, :, 1])

        # --- noobj conf loss ---
        _, uq = t2("uq")
        for b in range(2):
            vec.tensor_tensor(uq[:, :, b], P_cf[:, :, b], T_m,
                              op=ALU.subtract)
        sca.activation(uq, uq, ACT.Square)
        usum = t1("usum")
        vec.tensor_tensor(usum, uq[:, :, 0], uq[:, :, 1], op=ALU.add)
        nm = t1("nm", pool=tmp2)           # 0.5*(1-m)
        vec.tensor_scalar(nm, T_m, -0.5, 0.5, op0=ALU.mult, op1=ALU.add)

        # --- class loss ---
        dcl = tmp1.tile([P, R * 20], F32, tag="dcl", name="dcl")
        d3 = dcl[:].rearrange("p (r c) -> p r c", c=20)
        vec.tensor_tensor(d3, P_cls, T_cls, op=ALU.subtract)
        sca.activation(d3, d3, ACT.Square)
        q = t1("q")
        vec.tensor_reduce(q, d3, axis=mybir.AxisListType.X, op=ALU.add)

        # --- fused masked accumulations -> [128,1] partials ---
        tot = t1("tot")
        vec.tensor_tensor(tot, c, q, op=ALU.add)
        vec.scalar_tensor_tensor(tot, tot, 1.0, T_m, op0=ALU.bypass,
                                 op1=ALU.mult,
                                 accum_out=out_sb[:, 2 * k:2 * k + 1])
        vec.scalar_tensor_tensor(usum, usum, 1.0, nm, op0=ALU.bypass,
                                 op1=ALU.mult,
                                 accum_out=out_sb[:, 2 * k + 1:2 * k + 2])

    nc.sync.dma_start(out_ap, out_sb[:])


_CACHED = {}


def _get_compiled():
    if "nc" not in _CACHED:
        from contextlib import ExitStack
        nc = bacc.Bacc("TRN2", target_bir_lowering=False, debug=False,
                       enable_asserts=False, num_devices=NCORES)
        pred_t = nc.dram_tensor("pred", [P, RPP * NCH], F32,
                                kind="ExternalInput")
        targ_t = nc.dram_tensor("targ", [P, RPP * NCH], F32,
                                kind="ExternalInput")
        out_t = nc.dram_tensor("out", [P, 2 * NCHUNK], F32,
                               kind="ExternalOutput")
        with tile.TileContext(nc) as tc:
            with ExitStack() as ctx:
                build_loss_kernel(tc, out_t.ap(), pred_t.ap(), targ_t.ap(),
                                  ctx)
        nc.compile()
        _CACHED["nc"] = nc
    return _CACHED["nc"]


def _shard(arr):
    """[4096,14,14,30] -> list of 8 per-core [128, RPP*30] row-major blocks."""
    rows = np.ascontiguousarray(arr, dtype=np.float32).reshape(-1, NCH)
    per = ROWS_PER_CORE
    return [np.ascontiguousarray(
        rows[c * per:(c + 1) * per].reshape(P, RPP * NCH))
        for c in range(NCORES)]


def kernel(pred_tensor, target_tensor):
    nc = _get_compiled()
    preds = _shard(pred_tensor)
    targs = _shard(target_tensor)
    in_maps = [{"pred": preds[c], "targ": targs[c]} for c in range(NCORES)]
    res = run_bass_kernel_spmd(nc, in_maps, core_ids=list(range(NCORES)))
    total = 0.0
    for c in range(NCORES):
        total += res.results[c]["out"].astype(np.float64).sum()
    return np.float32(total / NB)
